# revision 8
# baseline (speedup 1.0000x reference)
"""Trainium2 Bass kernel for nn_CustomLoss_38062000177852.

Computes: CE(logits, tgt) + overlap_penalty(argmax(logits), sizes) for
logits [32,1024,1024] f32, tgt [32,1024] i32, sizes [32,1024] i32.

Sharding: batch dim (32) split 4-per-core across 8 NeuronCores (SPMD, one
Bass program, per-core input shards). Each core returns two partial sums
(ce_sum, overlap_count); host combines: loss = -ce/(B*T) + count/B.

Per-core layout: 4096 rows (b,t) -> 32 blocks of 128 rows. Row (b,t) lives
at partition p = t%128 of block k = b*8 + t//128 (flat row k*128+p).

Rev2: blocks are processed in GROUPS of 8 on the DVE so the row-max and
argmax each run as ONE instruction over [128, 8x1024] (8-needle
find_index8: needle j = max of segment j; match position = 1024*j + argmax
whp). This amortizes the fixed per-instruction DVE dispatch cost over 8x
the data; the DVE is the bottleneck engine (2 full passes over all logits
are unavoidable: reduce_max is 1x-only on DVE and no other engine can do a
free-axis max). exp/sum-exp rides on ACT, gathers + index algebra on
GpSimd, transposes/partition-sums on PE; the HBM stream saturates all 16
DMA engines (~358 GB/s) and everything else hides under the DVE passes.

Offset recurrence (reference scan): e_t = s_t + same_t*max(e_{t-1}-700, 0)
rewritten as e_t = max(e_{t-1} + a_t, b_t), a_t = same_t ? s_t-700 : -BIG,
b_t = s_t  -- a (max,+) linear scan, computed hierarchically: per-chunk scan
([32,128], t on free dim), chunk-map composition scan over 32 chunks, then
re-scan with per-chunk initial states. Exact in f32 (all values < 2^24).

Overlap count: pairs (t, t-d) need 700d < e_{t-d} - offs_t, so only d <= W
can overlap (adjacent d=1 provably never overlaps). Counted with a single
extended-tile compare per distance d in [2, W]: ext col 128+i holds the
NEXT chunk's col i, so in-chunk and chunk-boundary pairs share one compare.
"""
import numpy as np

import concourse.bacc as bacc
import concourse.bass as bass
import concourse.mybir as mybir
import concourse.tile as tile
from concourse import bass_utils
from concourse.masks import make_identity

f32 = mybir.dt.float32
i32 = mybir.dt.int32
u32 = mybir.dt.uint32
ALU = mybir.AluOpType
AX = mybir.AxisListType
ACTF = mybir.ActivationFunctionType

B, T, V = 32, 1024, 1024
NCORES = 8
BC = B // NCORES              # batches per core
NBLK = BC * (T // 128)        # 32 row-blocks per core
G = 8                         # blocks fused per DVE group instruction
NGRP = NBLK // G              # 4 groups
P = 128
TAKT = 700.0
BIG = 1.0e6                   # absorbing "minus infinity" for the scan input
NEG = -1.0e30                 # scan initial state
W = 6                         # max pair distance checked (d in [2, W])


def _build_program():
    nc = bacc.Bacc("TRN2", debug=False)

    lg = nc.dram_tensor("logits", [BC, T, V], f32, kind="ExternalInput")
    tg = nc.dram_tensor("tgt", [BC, T], i32, kind="ExternalInput")
    sz = nc.dram_tensor("sizes", [BC, V], i32, kind="ExternalInput")
    outd = nc.dram_tensor("out", [1, 2], f32, kind="ExternalOutput")

    lgf = lg.ap().rearrange("b t v -> (b t) v")          # [4096, 1024]
    lgflat = lg.ap().rearrange("b t v -> (b t v)").rearrange("(n o) -> n o", o=1)
    szflat = sz.ap().rearrange("b v -> (b v)").rearrange("(n o) -> n o", o=1)

    with tile.TileContext(nc) as tc:
        with (
            tc.tile_pool(name="big", bufs=1) as big,
            tc.tile_pool(name="sb", bufs=1) as sb,
            tc.tile_pool(name="scratch", bufs=2) as scratch,
            tc.tile_pool(name="ps", bufs=1, space="PSUM") as ps,
        ):
            # ---------------- constants / early independent work ----------
            ident = sb.tile([P, P], f32)
            make_identity(nc, ident)
            ones128 = sb.tile([P, 1], f32)
            nc.vector.memset(ones128[:], 1.0)
            ones11 = sb.tile([1, 1], f32)
            nc.vector.memset(ones11[:], 1.0)

            # tgt in [p, (b,c)] layout via strided DMA
            TGT = sb.tile([P, NBLK], i32)
            nc.sync.dma_start(
                out=TGT[:].rearrange("p (b c) -> p b c", b=BC),
                in_=tg.ap().rearrange("b (c p) -> p b c", p=P),
            )

            # x[tgt] gather: flat offset = (k*128+p)*1024 + tgt
            # (iota steps must fit int16, so build 128k+p then scale by 1024)
            OFB = sb.tile([P, NBLK], i32)
            nc.gpsimd.iota(OFB[:], pattern=[[P, NBLK]], base=0,
                           channel_multiplier=1)
            nc.vector.tensor_scalar(out=OFB[:], in0=OFB[:], scalar1=float(V),
                                    scalar2=None, op0=ALU.mult)
            OFFX = sb.tile([P, NBLK], i32)
            nc.vector.tensor_tensor(out=OFFX[:], in0=OFB[:], in1=TGT[:], op=ALU.add)
            # per-element gather: HW indirect DMA consumes one offset per
            # partition per instruction, so issue one column at a time
            XG = sb.tile([P, NBLK], f32)
            for k in range(NBLK):
                nc.gpsimd.indirect_dma_start(
                    out=XG[:, k:k + 1], out_offset=None, in_=lgflat,
                    in_offset=bass.IndirectOffsetOnAxis(ap=OFFX[:, k:k + 1], axis=0),
                )

            # b*1024 iota (batch id base for sizes gather / perm augmentation)
            BIOT = sb.tile([P, NBLK], i32)
            nc.gpsimd.iota(BIOT[:].rearrange("p (b c) -> p b c", b=BC),
                           pattern=[[T, BC], [0, NBLK // BC]], base=0,
                           channel_multiplier=0)
            # 1024*(k%8) iota: segment base of block k inside its group
            CIOT = sb.tile([P, NBLK], i32)
            nc.gpsimd.iota(CIOT[:].rearrange("p (g j) -> p g j", g=NGRP),
                           pattern=[[0, NGRP], [V, G]], base=0,
                           channel_multiplier=0)
            # COLC[p, k] = b(k)*1024 - 1024*(k%8): SIDX = IDXG + COLC
            COLC = sb.tile([P, NBLK], i32)
            nc.vector.tensor_tensor(out=COLC[:], in0=BIOT[:], in1=CIOT[:],
                                    op=ALU.subtract)

            # u*700 grid in [32, 128] layout (u = k*128 + f)
            UI = sb.tile([NBLK, P], i32)
            nc.gpsimd.iota(UI[:], pattern=[[1, P]], base=0, channel_multiplier=P)
            U700 = sb.tile([NBLK, P], f32)
            nc.vector.tensor_scalar(out=U700[:], in0=UI[:], scalar1=TAKT,
                                    scalar2=None, op0=ALU.mult)

            # ---------------- phase 1: stream logits, grouped x8 -----------
            # X8[g]: [128, 8, 1024]; partition p, segment j holds row
            # (8g+j)*128 + p of the per-core logits (4 KiB contiguous lines).
            X = big.tile([P, NBLK, V], f32)
            RMAX8 = sb.tile([P, NBLK], f32)
            IDX8 = sb.tile([P, NBLK, 8], u32)
            SUME = sb.tile([P, NBLK], f32)
            SIDX = sb.tile([P, NBLK], i32)
            SZG = sb.tile([P, NBLK], i32)

            # group g == batch g: rows (8g+j)*128+p == lg[g, j*128+p, :]
            xsrc = lg.ap().rearrange("b (j p) v -> b p j v", p=P)
            for g in range(NGRP):
                xg = X[:, g * G:(g + 1) * G, :]
                nc.sync.dma_start(out=xg, in_=xsrc[g])
                # one 1x DVE pass: row max of each of the 8 segments
                nc.vector.reduce_max(out=RMAX8[:, g * G:(g + 1) * G],
                                     in_=xg, axis=AX.X)
                # per-block find-index (needle broadcast x8, col 0 is argmax)
                for j in range(G):
                    k = g * G + j
                    nc.vector.max_index(
                        out=IDX8[:, k, :],
                        in_max=RMAX8[:, k:k + 1].to_broadcast([P, 8]),
                        in_values=X[:, k, :])
                # ACT: exp with per-block sum accumulate (accum must be
                # scalar per partition, so per-block slices)
                for j in range(G):
                    k = g * G + j
                    exps = scratch.tile([P, V], f32, tag="exps")
                    nc.scalar.activation(out=exps[:], in_=X[:, k, :],
                                         func=ACTF.Exp, bias=0.0, scale=1.0,
                                         accum_out=SUME[:, k:k + 1])
                # sizes[b, perm] gather offsets: b*1024 + perm  (gpsimd add)
                nc.gpsimd.tensor_tensor(
                    out=SIDX[:, g * G:(g + 1) * G],
                    in0=IDX8[:, g * G:(g + 1) * G, 0].bitcast(i32),
                    in1=BIOT[:, g * G:(g + 1) * G], op=ALU.add)
                for j in range(G):
                    k = g * G + j
                    nc.gpsimd.indirect_dma_start(
                        out=SZG[:, k:k + 1], out_offset=None, in_=szflat,
                        in_offset=bass.IndirectOffsetOnAxis(
                            ap=SIDX[:, k:k + 1], axis=0),
                    )

            # ---------------- CE partial -----------------------------------
            LSE = sb.tile([P, NBLK], f32)
            nc.scalar.activation(out=LSE[:], in_=SUME[:], func=ACTF.Ln,
                                 bias=0.0, scale=1.0)
            CET = sb.tile([P, NBLK], f32)
            nc.vector.tensor_tensor(out=CET[:], in0=XG[:], in1=LSE[:],
                                    op=ALU.subtract)
            CEcol = sb.tile([P, 1], f32)
            nc.vector.reduce_sum(out=CEcol[:], in_=CET[:], axis=AX.X)

            # ---------------- phase 2: scan + pair count -------------------
            SZF = sb.tile([P, NBLK], f32)
            nc.vector.tensor_copy(out=SZF[:], in_=SZG[:])
            PERMA = sb.tile([P, NBLK], f32)
            nc.vector.tensor_copy(out=PERMA[:], in_=SIDX[:])  # perm + b*1024

            # transposes to [32, 128] (t on free dim within chunk)
            PT1 = ps.tile([NBLK, P], f32, space="PSUM")
            nc.tensor.transpose(out=PT1[:], in_=PERMA[:], identity=ident[:])
            P32 = sb.tile([NBLK, P], f32)
            nc.vector.tensor_copy(out=P32[:], in_=PT1[:])
            PT2 = ps.tile([NBLK, P], f32, space="PSUM")
            nc.tensor.transpose(out=PT2[:], in_=SZF[:], identity=ident[:])
            S32 = sb.tile([NBLK, P], f32)
            nc.vector.tensor_copy(out=S32[:], in_=PT2[:])

            # prev-chunk shift (row k <- row k-1; row 0 wraps to row 31 whose
            # contribution always cancels via the b*1024 augmentation)
            shmask = [31] + list(range(31))
            SHP = sb.tile([NBLK, P], f32)
            nc.vector.stream_shuffle(out=SHP[:], in_=P32[:], mask=shmask)

            # same-station flags vs previous slot (aug makes cross-batch False)
            SAME = sb.tile([NBLK, P], f32)
            nc.vector.tensor_tensor(out=SAME[:, 1:P], in0=P32[:, 1:P],
                                    in1=P32[:, 0:P - 1], op=ALU.is_equal)
            nc.vector.tensor_tensor(out=SAME[:, 0:1], in0=P32[:, 0:1],
                                    in1=SHP[:, P - 1:P], op=ALU.is_equal)

            # a_t = same ? s_t - 700 : -BIG   (exact integer algebra in f32)
            A32 = sb.tile([NBLK, P], f32)
            nc.vector.tensor_scalar(out=A32[:], in0=S32[:], scalar1=BIG - TAKT,
                                    scalar2=None, op0=ALU.add)
            nc.vector.tensor_tensor(out=A32[:], in0=A32[:], in1=SAME[:],
                                    op=ALU.mult)
            nc.vector.tensor_scalar(out=A32[:], in0=A32[:], scalar1=BIG,
                                    scalar2=None, op0=ALU.subtract)

            # level-1 scan within chunks
            E1 = sb.tile([NBLK, P], f32)
            nc.vector.tensor_tensor_scan(out=E1[:], data0=A32[:], data1=S32[:],
                                         initial=NEG, op0=ALU.add, op1=ALU.max)
            ACOL = sb.tile([NBLK, 1], f32)
            nc.vector.reduce_sum(out=ACOL[:], in_=A32[:], axis=AX.X)
            BCOL = sb.tile([NBLK, 1], f32)
            nc.vector.tensor_copy(out=BCOL[:], in_=E1[:, P - 1:P])

            # level-2 scan across the 32 chunk maps (cols -> rows via matmul)
            PA = ps.tile([1, NBLK], f32, space="PSUM")
            nc.tensor.matmul(out=PA[:], lhsT=ACOL[:],
                             rhs=ident[0:NBLK, 0:NBLK], start=True, stop=True)
            PB = ps.tile([1, NBLK], f32, space="PSUM")
            nc.tensor.matmul(out=PB[:], lhsT=BCOL[:],
                             rhs=ident[0:NBLK, 0:NBLK], start=True, stop=True)
            ASB = sb.tile([1, NBLK], f32)
            nc.vector.tensor_copy(out=ASB[:], in_=PA[:])
            BSB = sb.tile([1, NBLK], f32)
            nc.vector.tensor_copy(out=BSB[:], in_=PB[:])
            S2 = sb.tile([1, NBLK], f32)
            nc.vector.tensor_tensor_scan(out=S2[:], data0=ASB[:],
                                         data1=BSB[:], initial=NEG,
                                         op0=ALU.add, op1=ALU.max)
            EINR = sb.tile([1, NBLK], f32)
            nc.vector.memset(EINR[:, 0:1], NEG)
            nc.vector.tensor_copy(out=EINR[:, 1:NBLK], in_=S2[:, 0:NBLK - 1])
            PEIN = ps.tile([NBLK, 1], f32, space="PSUM")
            nc.tensor.matmul(out=PEIN[:], lhsT=EINR[:], rhs=ones11[:],
                             start=True, stop=True)
            EIN = sb.tile([NBLK, 1], f32)
            nc.vector.tensor_copy(out=EIN[:], in_=PEIN[:])

            # level-3: exact e per slot; xe = 700u + e, xs = xe - s
            E = sb.tile([NBLK, P], f32)
            nc.vector.tensor_tensor_scan(out=E[:], data0=A32[:], data1=S32[:],
                                         initial=EIN[:], op0=ALU.add, op1=ALU.max)
            XE = sb.tile([NBLK, P], f32)
            nc.vector.tensor_tensor(out=XE[:], in0=E[:], in1=U700[:], op=ALU.add)
            XS = sb.tile([NBLK, P], f32)
            nc.vector.tensor_tensor(out=XS[:], in0=XE[:], in1=S32[:],
                                    op=ALU.subtract)

            # extended tiles: col 128+i = NEXT chunk's col i (row 31 wraps to
            # row 0: cross-batch, cancels via the b*1024 augmentation)
            nxmask = list(range(1, NBLK)) + [0]
            EXTW = W  # ext columns needed: distances up to W
            PX = sb.tile([NBLK, P + EXTW], f32)
            SX = sb.tile([NBLK, P + EXTW], f32)
            EX = sb.tile([NBLK, P + EXTW], f32)
            SHN = sb.tile([NBLK, P], f32)
            nc.vector.tensor_copy(out=PX[:, 0:P], in_=P32[:])
            nc.vector.tensor_copy(out=SX[:, 0:P], in_=XS[:])
            nc.vector.tensor_copy(out=EX[:, 0:P], in_=XE[:])
            nc.vector.stream_shuffle(out=SHN[:], in_=P32[:], mask=nxmask)
            nc.vector.tensor_copy(out=PX[:, P:P + EXTW], in_=SHN[:, 0:EXTW])
            nc.vector.stream_shuffle(out=SHN[:], in_=XS[:], mask=nxmask)
            nc.vector.tensor_copy(out=SX[:, P:P + EXTW], in_=SHN[:, 0:EXTW])
            nc.vector.stream_shuffle(out=SHN[:], in_=XE[:], mask=nxmask)
            nc.vector.tensor_copy(out=EX[:, P:P + EXTW], in_=SHN[:, 0:EXTW])

            # pair count: for d in [2, W], one unified compare per d:
            # t = ext col j+d, t' = own col j, j in [0, 128)
            ACC = sb.tile([NBLK, W - 1], f32)
            CJ = scratch.tile([NBLK, P], f32, tag="cj")
            C2 = scratch.tile([NBLK, P], f32, tag="c2")
            for d in range(2, W + 1):
                nc.vector.tensor_tensor(out=CJ[:], in0=PX[:, d:d + P],
                                        in1=P32[:], op=ALU.is_equal)
                nc.vector.tensor_tensor(out=C2[:], in0=SX[:, d:d + P],
                                        in1=XE[:], op=ALU.is_lt)
                nc.vector.tensor_tensor(out=CJ[:], in0=CJ[:], in1=C2[:],
                                        op=ALU.mult)
                nc.vector.tensor_tensor(out=C2[:], in0=EX[:, d:d + P],
                                        in1=XS[:], op=ALU.is_gt)
                nc.vector.tensor_tensor(out=CJ[:], in0=CJ[:], in1=C2[:],
                                        op=ALU.mult)
                nc.vector.reduce_sum(out=ACC[:, d - 2:d - 1], in_=CJ[:],
                                     axis=AX.X)

            CNT = sb.tile([NBLK, 1], f32)
            nc.vector.reduce_sum(out=CNT[:], in_=ACC[:], axis=AX.X)

            # ---------------- partial sums out -----------------------------
            PSC = ps.tile([1, 2], f32, space="PSUM")
            nc.tensor.matmul(out=PSC[:, 0:1], lhsT=CEcol[:], rhs=ones128[:],
                             start=True, stop=True)
            nc.tensor.matmul(out=PSC[:, 1:2], lhsT=CNT[:],
                             rhs=ones128[0:NBLK, :], start=True, stop=True)
            OUTSB = sb.tile([1, 2], f32)
            nc.vector.tensor_copy(out=OUTSB[:], in_=PSC[:])
            nc.sync.dma_start(out=outd.ap(), in_=OUTSB[:])

    nc.compile()
    return nc


_NC_CACHE = None
LAST_RESULTS = None  # test.py inspects this for HW exec time when BASS_TRACE=1


def _get_program():
    global _NC_CACHE
    if _NC_CACHE is None:
        _NC_CACHE = _build_program()
    return _NC_CACHE


def kernel(logits: np.ndarray, tgt: np.ndarray, sizes: np.ndarray) -> np.ndarray:
    logits = np.ascontiguousarray(np.asarray(logits, np.float32))
    tgt = np.ascontiguousarray(np.asarray(tgt, np.int32))
    sizes = np.ascontiguousarray(np.asarray(sizes, np.int32))
    assert logits.shape == (B, T, V)

    nc = _get_program()
    in_maps = []
    for i in range(NCORES):
        s = slice(i * BC, (i + 1) * BC)
        in_maps.append({
            "logits": logits[s],
            "tgt": tgt[s],
            "sizes": sizes[s],
        })
    res = bass_utils.run_bass_kernel_spmd(nc, in_maps, core_ids=list(range(NCORES)))
    global LAST_RESULTS
    LAST_RESULTS = res
    ce_sum = 0.0
    cnt_sum = 0.0
    for r in res.results:
        o = r["out"]
        ce_sum += float(o[0, 0])
        cnt_sum += float(o[0, 1])
    loss = -(ce_sum) / (B * T) + cnt_sum / B
    return np.asarray(loss, dtype=np.float32)


# revision 9
# speedup vs baseline: 1.0231x; 1.0231x over previous
"""Trainium2 Bass kernel for nn_CustomLoss_38062000177852.

Computes: CE(logits, tgt) + overlap_penalty(argmax(logits), sizes) for
logits [32,1024,1024] f32, tgt [32,1024] i32, sizes [32,1024] i32.

Sharding: batch dim (32) split 4-per-core across 8 NeuronCores (SPMD, one
Bass program, per-core input shards). Each core returns two partial sums
(ce_sum, overlap_count); host combines: loss = -ce/(B*T) + count/B.

Per-core layout: 4096 rows (b,t) -> 32 blocks of 128 rows. Row (b,t) lives
at partition p = t%128 of block k = b*8 + t//128 (flat row k*128+p).

Rev2: blocks are processed in GROUPS of 8 on the DVE so the row-max and
argmax each run as ONE instruction over [128, 8x1024] (8-needle
find_index8: needle j = max of segment j; match position = 1024*j + argmax
whp). This amortizes the fixed per-instruction DVE dispatch cost over 8x
the data; the DVE is the bottleneck engine (2 full passes over all logits
are unavoidable: reduce_max is 1x-only on DVE and no other engine can do a
free-axis max). exp/sum-exp rides on ACT, gathers + index algebra on
GpSimd, transposes/partition-sums on PE; the HBM stream saturates all 16
DMA engines (~358 GB/s) and everything else hides under the DVE passes.

Offset recurrence (reference scan): e_t = s_t + same_t*max(e_{t-1}-700, 0)
rewritten as e_t = max(e_{t-1} + a_t, b_t), a_t = same_t ? s_t-700 : -BIG,
b_t = s_t  -- a (max,+) linear scan, computed hierarchically: per-chunk scan
([32,128], t on free dim), chunk-map composition scan over 32 chunks, then
re-scan with per-chunk initial states. Exact in f32 (all values < 2^24).

Overlap count: pairs (t, t-d) need 700d < e_{t-d} - offs_t, so only d <= W
can overlap (adjacent d=1 provably never overlaps). Counted with a single
extended-tile compare per distance d in [2, W]: ext col 128+i holds the
NEXT chunk's col i, so in-chunk and chunk-boundary pairs share one compare.
"""
import numpy as np

import concourse.bacc as bacc
import concourse.bass as bass
import concourse.mybir as mybir
import concourse.tile as tile
from concourse import bass_utils
from concourse.masks import make_identity

f32 = mybir.dt.float32
i32 = mybir.dt.int32
u32 = mybir.dt.uint32
ALU = mybir.AluOpType
AX = mybir.AxisListType
ACTF = mybir.ActivationFunctionType

B, T, V = 32, 1024, 1024
NCORES = 8
BC = B // NCORES              # batches per core
NBLK = BC * (T // 128)        # 32 row-blocks per core
G = 8                         # blocks fused per DVE group instruction
NGRP = NBLK // G              # 4 groups
P = 128
TAKT = 700.0
BIG = 1.0e6                   # absorbing "minus infinity" for the scan input
NEG = -1.0e30                 # scan initial state
W = 6                         # max pair distance checked (d in [2, W])


def _build_program():
    nc = bacc.Bacc("TRN2", debug=False)

    lg = nc.dram_tensor("logits", [BC, T, V], f32, kind="ExternalInput")
    tg = nc.dram_tensor("tgt", [BC, T], i32, kind="ExternalInput")
    sz = nc.dram_tensor("sizes", [BC, V], i32, kind="ExternalInput")
    outd = nc.dram_tensor("out", [1, 2], f32, kind="ExternalOutput")

    lgf = lg.ap().rearrange("b t v -> (b t) v")          # [4096, 1024]
    lgflat = lg.ap().rearrange("b t v -> (b t v)").rearrange("(n o) -> n o", o=1)
    szflat = sz.ap().rearrange("b v -> (b v)").rearrange("(n o) -> n o", o=1)

    with tile.TileContext(nc) as tc:
        with (
            tc.tile_pool(name="big", bufs=1) as big,
            tc.tile_pool(name="sb", bufs=1) as sb,
            tc.tile_pool(name="scratch", bufs=2) as scratch,
            tc.tile_pool(name="ps", bufs=1, space="PSUM") as ps,
        ):
            # ---------------- constants / early independent work ----------
            ident = sb.tile([P, P], f32)
            make_identity(nc, ident)
            ones128 = sb.tile([P, 1], f32)
            nc.vector.memset(ones128[:], 1.0)
            ones11 = sb.tile([1, 1], f32)
            nc.vector.memset(ones11[:], 1.0)

            # tgt in [p, (b,c)] layout via strided DMA
            TGT = sb.tile([P, NBLK], i32)
            nc.sync.dma_start(
                out=TGT[:].rearrange("p (b c) -> p b c", b=BC),
                in_=tg.ap().rearrange("b (c p) -> p b c", p=P),
            )

            # x[tgt] gather: flat offset = (k*128+p)*1024 + tgt
            # (iota steps must fit int16, so build 128k+p then scale by 1024)
            OFB = sb.tile([P, NBLK], i32)
            nc.gpsimd.iota(OFB[:], pattern=[[P, NBLK]], base=0,
                           channel_multiplier=1)
            nc.vector.tensor_scalar(out=OFB[:], in0=OFB[:], scalar1=float(V),
                                    scalar2=None, op0=ALU.mult)
            OFFX = sb.tile([P, NBLK], i32)
            nc.vector.tensor_tensor(out=OFFX[:], in0=OFB[:], in1=TGT[:], op=ALU.add)
            # per-element gather: HW indirect DMA consumes one offset per
            # partition per instruction, so issue one column at a time
            XG = sb.tile([P, NBLK], f32)
            for k in range(NBLK):
                nc.gpsimd.indirect_dma_start(
                    out=XG[:, k:k + 1], out_offset=None, in_=lgflat,
                    in_offset=bass.IndirectOffsetOnAxis(ap=OFFX[:, k:k + 1], axis=0),
                )

            # b*1024 iota (batch id base for sizes gather / perm augmentation)
            BIOT = sb.tile([P, NBLK], i32)
            nc.gpsimd.iota(BIOT[:].rearrange("p (b c) -> p b c", b=BC),
                           pattern=[[T, BC], [0, NBLK // BC]], base=0,
                           channel_multiplier=0)
            # 1024*(k%8) iota: segment base of block k inside its group
            CIOT = sb.tile([P, NBLK], i32)
            nc.gpsimd.iota(CIOT[:].rearrange("p (g j) -> p g j", g=NGRP),
                           pattern=[[0, NGRP], [V, G]], base=0,
                           channel_multiplier=0)
            # COLC[p, k] = b(k)*1024 - 1024*(k%8): SIDX = IDXG + COLC
            COLC = sb.tile([P, NBLK], i32)
            nc.vector.tensor_tensor(out=COLC[:], in0=BIOT[:], in1=CIOT[:],
                                    op=ALU.subtract)

            # u*700 grid in [32, 128] layout (u = k*128 + f)
            UI = sb.tile([NBLK, P], i32)
            nc.gpsimd.iota(UI[:], pattern=[[1, P]], base=0, channel_multiplier=P)
            U700 = sb.tile([NBLK, P], f32)
            nc.vector.tensor_scalar(out=U700[:], in0=UI[:], scalar1=TAKT,
                                    scalar2=None, op0=ALU.mult)

            # ---------------- phase 1: stream logits, grouped x8 -----------
            # X8[g]: [128, 8, 1024]; partition p, segment j holds row
            # (8g+j)*128 + p of the per-core logits (4 KiB contiguous lines).
            X = big.tile([P, NBLK, V], f32)
            RMAX8 = sb.tile([P, NBLK], f32)
            IDX8 = sb.tile([P, NBLK, 8], u32)
            SUME = sb.tile([P, NBLK], f32)
            SIDX = sb.tile([P, NBLK], i32)
            SZG = sb.tile([P, NBLK], i32)

            # per-block streaming: 32 DMAs round-robin across all 16 DMA
            # engines (one coarse group DMA only engages a few queues and
            # starves the DVE pipeline start)
            for k in range(NBLK):
                xk = X[:, k, :]
                nc.sync.dma_start(out=xk, in_=lgf[k * P:(k + 1) * P, :])
                nc.vector.reduce_max(out=RMAX8[:, k:k + 1], in_=xk, axis=AX.X)
                nc.vector.max_index(
                    out=IDX8[:, k, :],
                    in_max=RMAX8[:, k:k + 1].to_broadcast([P, 8]),
                    in_values=xk)
                exps = scratch.tile([P, V], f32, tag="exps")
                nc.scalar.activation(out=exps[:], in_=xk,
                                     func=ACTF.Exp, bias=0.0, scale=1.0,
                                     accum_out=SUME[:, k:k + 1])
                # sizes[b, perm] gather offset: b*1024 + perm  (gpsimd add)
                nc.gpsimd.tensor_tensor(
                    out=SIDX[:, k:k + 1],
                    in0=IDX8[:, k, 0:1].bitcast(i32),
                    in1=BIOT[:, k:k + 1], op=ALU.add)
                nc.gpsimd.indirect_dma_start(
                    out=SZG[:, k:k + 1], out_offset=None, in_=szflat,
                    in_offset=bass.IndirectOffsetOnAxis(
                        ap=SIDX[:, k:k + 1], axis=0),
                )

            # ---------------- CE partial -----------------------------------
            LSE = sb.tile([P, NBLK], f32)
            nc.scalar.activation(out=LSE[:], in_=SUME[:], func=ACTF.Ln,
                                 bias=0.0, scale=1.0)
            CET = sb.tile([P, NBLK], f32)
            nc.vector.tensor_tensor(out=CET[:], in0=XG[:], in1=LSE[:],
                                    op=ALU.subtract)
            CEcol = sb.tile([P, 1], f32)
            nc.vector.reduce_sum(out=CEcol[:], in_=CET[:], axis=AX.X)

            # ---------------- phase 2: scan + pair count -------------------
            SZF = sb.tile([P, NBLK], f32)
            nc.vector.tensor_copy(out=SZF[:], in_=SZG[:])
            PERMA = sb.tile([P, NBLK], f32)
            nc.vector.tensor_copy(out=PERMA[:], in_=SIDX[:])  # perm + b*1024

            # transposes to [32, 128] (t on free dim within chunk)
            PT1 = ps.tile([NBLK, P], f32, space="PSUM")
            nc.tensor.transpose(out=PT1[:], in_=PERMA[:], identity=ident[:])
            P32 = sb.tile([NBLK, P], f32)
            nc.vector.tensor_copy(out=P32[:], in_=PT1[:])
            PT2 = ps.tile([NBLK, P], f32, space="PSUM")
            nc.tensor.transpose(out=PT2[:], in_=SZF[:], identity=ident[:])
            S32 = sb.tile([NBLK, P], f32)
            nc.vector.tensor_copy(out=S32[:], in_=PT2[:])

            # prev-chunk shift (row k <- row k-1; row 0 wraps to row 31 whose
            # contribution always cancels via the b*1024 augmentation)
            shmask = [31] + list(range(31))
            SHP = sb.tile([NBLK, P], f32)
            nc.vector.stream_shuffle(out=SHP[:], in_=P32[:], mask=shmask)

            # same-station flags vs previous slot (aug makes cross-batch False)
            SAME = sb.tile([NBLK, P], f32)
            nc.vector.tensor_tensor(out=SAME[:, 1:P], in0=P32[:, 1:P],
                                    in1=P32[:, 0:P - 1], op=ALU.is_equal)
            nc.vector.tensor_tensor(out=SAME[:, 0:1], in0=P32[:, 0:1],
                                    in1=SHP[:, P - 1:P], op=ALU.is_equal)

            # a_t = same ? s_t - 700 : -BIG   (exact integer algebra in f32)
            A32 = sb.tile([NBLK, P], f32)
            nc.vector.tensor_scalar(out=A32[:], in0=S32[:], scalar1=BIG - TAKT,
                                    scalar2=None, op0=ALU.add)
            nc.vector.tensor_tensor(out=A32[:], in0=A32[:], in1=SAME[:],
                                    op=ALU.mult)
            nc.vector.tensor_scalar(out=A32[:], in0=A32[:], scalar1=BIG,
                                    scalar2=None, op0=ALU.subtract)

            # level-1 scan within chunks
            E1 = sb.tile([NBLK, P], f32)
            nc.vector.tensor_tensor_scan(out=E1[:], data0=A32[:], data1=S32[:],
                                         initial=NEG, op0=ALU.add, op1=ALU.max)
            ACOL = sb.tile([NBLK, 1], f32)
            nc.vector.reduce_sum(out=ACOL[:], in_=A32[:], axis=AX.X)
            BCOL = sb.tile([NBLK, 1], f32)
            nc.vector.tensor_copy(out=BCOL[:], in_=E1[:, P - 1:P])

            # level-2 scan across the 32 chunk maps (cols -> rows via matmul)
            PA = ps.tile([1, NBLK], f32, space="PSUM")
            nc.tensor.matmul(out=PA[:], lhsT=ACOL[:],
                             rhs=ident[0:NBLK, 0:NBLK], start=True, stop=True)
            PB = ps.tile([1, NBLK], f32, space="PSUM")
            nc.tensor.matmul(out=PB[:], lhsT=BCOL[:],
                             rhs=ident[0:NBLK, 0:NBLK], start=True, stop=True)
            ASB = sb.tile([1, NBLK], f32)
            nc.vector.tensor_copy(out=ASB[:], in_=PA[:])
            BSB = sb.tile([1, NBLK], f32)
            nc.vector.tensor_copy(out=BSB[:], in_=PB[:])
            S2 = sb.tile([1, NBLK], f32)
            nc.vector.tensor_tensor_scan(out=S2[:], data0=ASB[:],
                                         data1=BSB[:], initial=NEG,
                                         op0=ALU.add, op1=ALU.max)
            EINR = sb.tile([1, NBLK], f32)
            nc.vector.memset(EINR[:, 0:1], NEG)
            nc.vector.tensor_copy(out=EINR[:, 1:NBLK], in_=S2[:, 0:NBLK - 1])
            PEIN = ps.tile([NBLK, 1], f32, space="PSUM")
            nc.tensor.matmul(out=PEIN[:], lhsT=EINR[:], rhs=ones11[:],
                             start=True, stop=True)
            EIN = sb.tile([NBLK, 1], f32)
            nc.vector.tensor_copy(out=EIN[:], in_=PEIN[:])

            # level-3: exact e per slot; xe = 700u + e, xs = xe - s
            E = sb.tile([NBLK, P], f32)
            nc.vector.tensor_tensor_scan(out=E[:], data0=A32[:], data1=S32[:],
                                         initial=EIN[:], op0=ALU.add, op1=ALU.max)
            XE = sb.tile([NBLK, P], f32)
            nc.vector.tensor_tensor(out=XE[:], in0=E[:], in1=U700[:], op=ALU.add)
            XS = sb.tile([NBLK, P], f32)
            nc.vector.tensor_tensor(out=XS[:], in0=XE[:], in1=S32[:],
                                    op=ALU.subtract)

            # extended tiles: col 128+i = NEXT chunk's col i (row 31 wraps to
            # row 0: cross-batch, cancels via the b*1024 augmentation)
            nxmask = list(range(1, NBLK)) + [0]
            EXTW = W  # ext columns needed: distances up to W
            PX = sb.tile([NBLK, P + EXTW], f32)
            SX = sb.tile([NBLK, P + EXTW], f32)
            EX = sb.tile([NBLK, P + EXTW], f32)
            SHN = sb.tile([NBLK, P], f32)
            nc.vector.tensor_copy(out=PX[:, 0:P], in_=P32[:])
            nc.vector.tensor_copy(out=SX[:, 0:P], in_=XS[:])
            nc.vector.tensor_copy(out=EX[:, 0:P], in_=XE[:])
            nc.vector.stream_shuffle(out=SHN[:], in_=P32[:], mask=nxmask)
            nc.vector.tensor_copy(out=PX[:, P:P + EXTW], in_=SHN[:, 0:EXTW])
            nc.vector.stream_shuffle(out=SHN[:], in_=XS[:], mask=nxmask)
            nc.vector.tensor_copy(out=SX[:, P:P + EXTW], in_=SHN[:, 0:EXTW])
            nc.vector.stream_shuffle(out=SHN[:], in_=XE[:], mask=nxmask)
            nc.vector.tensor_copy(out=EX[:, P:P + EXTW], in_=SHN[:, 0:EXTW])

            # pair count: for d in [2, W], one unified compare per d:
            # t = ext col j+d, t' = own col j, j in [0, 128)
            ACC = sb.tile([NBLK, W - 1], f32)
            CJ = scratch.tile([NBLK, P], f32, tag="cj")
            C2 = scratch.tile([NBLK, P], f32, tag="c2")
            for d in range(2, W + 1):
                nc.vector.tensor_tensor(out=CJ[:], in0=PX[:, d:d + P],
                                        in1=P32[:], op=ALU.is_equal)
                nc.vector.tensor_tensor(out=C2[:], in0=SX[:, d:d + P],
                                        in1=XE[:], op=ALU.is_lt)
                nc.vector.tensor_tensor(out=CJ[:], in0=CJ[:], in1=C2[:],
                                        op=ALU.mult)
                nc.vector.tensor_tensor(out=C2[:], in0=EX[:, d:d + P],
                                        in1=XS[:], op=ALU.is_gt)
                nc.vector.tensor_tensor(out=CJ[:], in0=CJ[:], in1=C2[:],
                                        op=ALU.mult)
                nc.vector.reduce_sum(out=ACC[:, d - 2:d - 1], in_=CJ[:],
                                     axis=AX.X)

            CNT = sb.tile([NBLK, 1], f32)
            nc.vector.reduce_sum(out=CNT[:], in_=ACC[:], axis=AX.X)

            # ---------------- partial sums out -----------------------------
            PSC = ps.tile([1, 2], f32, space="PSUM")
            nc.tensor.matmul(out=PSC[:, 0:1], lhsT=CEcol[:], rhs=ones128[:],
                             start=True, stop=True)
            nc.tensor.matmul(out=PSC[:, 1:2], lhsT=CNT[:],
                             rhs=ones128[0:NBLK, :], start=True, stop=True)
            OUTSB = sb.tile([1, 2], f32)
            nc.vector.tensor_copy(out=OUTSB[:], in_=PSC[:])
            nc.sync.dma_start(out=outd.ap(), in_=OUTSB[:])

    nc.compile()
    return nc


_NC_CACHE = None
LAST_RESULTS = None  # test.py inspects this for HW exec time when BASS_TRACE=1


def _get_program():
    global _NC_CACHE
    if _NC_CACHE is None:
        _NC_CACHE = _build_program()
    return _NC_CACHE


def kernel(logits: np.ndarray, tgt: np.ndarray, sizes: np.ndarray) -> np.ndarray:
    logits = np.ascontiguousarray(np.asarray(logits, np.float32))
    tgt = np.ascontiguousarray(np.asarray(tgt, np.int32))
    sizes = np.ascontiguousarray(np.asarray(sizes, np.int32))
    assert logits.shape == (B, T, V)

    nc = _get_program()
    in_maps = []
    for i in range(NCORES):
        s = slice(i * BC, (i + 1) * BC)
        in_maps.append({
            "logits": logits[s],
            "tgt": tgt[s],
            "sizes": sizes[s],
        })
    res = bass_utils.run_bass_kernel_spmd(nc, in_maps, core_ids=list(range(NCORES)))
    global LAST_RESULTS
    LAST_RESULTS = res
    ce_sum = 0.0
    cnt_sum = 0.0
    for r in res.results:
        o = r["out"]
        ce_sum += float(o[0, 0])
        cnt_sum += float(o[0, 1])
    loss = -(ce_sum) / (B * T) + cnt_sum / B
    return np.asarray(loss, dtype=np.float32)


# revision 11
# speedup vs baseline: 1.0246x; 1.0015x over previous
"""Trainium2 Bass kernel for nn_CustomLoss_38062000177852.

Computes: CE(logits, tgt) + overlap_penalty(argmax(logits), sizes) for
logits [32,1024,1024] f32, tgt [32,1024] i32, sizes [32,1024] i32.

Sharding: batch dim (32) split 4-per-core across 8 NeuronCores (SPMD, one
Bass program, per-core input shards). Each core returns two partial sums
(ce_sum, overlap_count); host combines: loss = -ce/(B*T) + count/B.

Per-core layout: 4096 rows (b,t) -> 32 blocks of 128 rows. Row (b,t) lives
at partition p = t%128 of block k = b*8 + t//128 (flat row k*128+p).

Rev2: blocks are processed in GROUPS of 8 on the DVE so the row-max and
argmax each run as ONE instruction over [128, 8x1024] (8-needle
find_index8: needle j = max of segment j; match position = 1024*j + argmax
whp). This amortizes the fixed per-instruction DVE dispatch cost over 8x
the data; the DVE is the bottleneck engine (2 full passes over all logits
are unavoidable: reduce_max is 1x-only on DVE and no other engine can do a
free-axis max). exp/sum-exp rides on ACT, gathers + index algebra on
GpSimd, transposes/partition-sums on PE; the HBM stream saturates all 16
DMA engines (~358 GB/s) and everything else hides under the DVE passes.

Offset recurrence (reference scan): e_t = s_t + same_t*max(e_{t-1}-700, 0)
rewritten as e_t = max(e_{t-1} + a_t, b_t), a_t = same_t ? s_t-700 : -BIG,
b_t = s_t  -- a (max,+) linear scan, computed hierarchically: per-chunk scan
([32,128], t on free dim), chunk-map composition scan over 32 chunks, then
re-scan with per-chunk initial states. Exact in f32 (all values < 2^24).

Overlap count: pairs (t, t-d) need 700d < e_{t-d} - offs_t, so only d <= W
can overlap (adjacent d=1 provably never overlaps). Counted with a single
extended-tile compare per distance d in [2, W]: ext col 128+i holds the
NEXT chunk's col i, so in-chunk and chunk-boundary pairs share one compare.
"""
import numpy as np

import concourse.bacc as bacc
import concourse.bass as bass
import concourse.mybir as mybir
import concourse.tile as tile
from concourse import bass_utils
from concourse.masks import make_identity

f32 = mybir.dt.float32
i32 = mybir.dt.int32
u32 = mybir.dt.uint32
ALU = mybir.AluOpType
AX = mybir.AxisListType
ACTF = mybir.ActivationFunctionType

B, T, V = 32, 1024, 1024
NCORES = 8
BC = B // NCORES              # batches per core
NBLK = BC * (T // 128)        # 32 row-blocks per core
G = 8                         # blocks fused per DVE group instruction
NGRP = NBLK // G              # 4 groups
P = 128
TAKT = 700.0
BIG = 1.0e6                   # absorbing "minus infinity" for the scan input
NEG = -1.0e30                 # scan initial state
W = 6                         # max pair distance checked (d in [2, W])


def _build_program():
    nc = bacc.Bacc("TRN2", debug=False)

    lg = nc.dram_tensor("logits", [BC, T, V], f32, kind="ExternalInput")
    tg = nc.dram_tensor("tgt", [BC, T], i32, kind="ExternalInput")
    sz = nc.dram_tensor("sizes", [BC, V], i32, kind="ExternalInput")
    outd = nc.dram_tensor("out", [1, 2], f32, kind="ExternalOutput")

    lgf = lg.ap().rearrange("b t v -> (b t) v")          # [4096, 1024]
    lgflat = lg.ap().rearrange("b t v -> (b t v)").rearrange("(n o) -> n o", o=1)
    szflat = sz.ap().rearrange("b v -> (b v)").rearrange("(n o) -> n o", o=1)

    with tile.TileContext(nc) as tc:
        with (
            tc.tile_pool(name="big", bufs=1) as big,
            tc.tile_pool(name="sb", bufs=1) as sb,
            tc.tile_pool(name="scratch", bufs=2) as scratch,
            tc.tile_pool(name="ps", bufs=1, space="PSUM") as ps,
        ):
            # ---------------- constants / early independent work ----------
            ident = sb.tile([P, P], f32)
            make_identity(nc, ident)
            ones128 = sb.tile([P, 1], f32)
            nc.vector.memset(ones128[:], 1.0)
            ones11 = sb.tile([1, 1], f32)
            nc.vector.memset(ones11[:], 1.0)

            # tgt in [p, (b,c)] layout via strided DMA
            TGT = sb.tile([P, NBLK], i32)
            nc.sync.dma_start(
                out=TGT[:].rearrange("p (b c) -> p b c", b=BC),
                in_=tg.ap().rearrange("b (c p) -> p b c", p=P),
            )

            # x[tgt] gather: flat offset = (k*128+p)*1024 + tgt
            # (iota steps must fit int16, so build 128k+p then scale by 1024)
            OFB = sb.tile([P, NBLK], i32)
            nc.gpsimd.iota(OFB[:], pattern=[[P, NBLK]], base=0,
                           channel_multiplier=1)
            nc.vector.tensor_scalar(out=OFB[:], in0=OFB[:], scalar1=float(V),
                                    scalar2=None, op0=ALU.mult)
            OFFX = sb.tile([P, NBLK], i32)
            nc.vector.tensor_tensor(out=OFFX[:], in0=OFB[:], in1=TGT[:], op=ALU.add)
            # per-element gather: HW indirect DMA consumes one offset per
            # partition per instruction, so one column at a time; the 32
            # columns are issued inside the phase-1 loop to fill GpSimd idle
            # slots between the find8-gated sizes gathers
            XG = sb.tile([P, NBLK], f32)

            # b*1024 iota (batch id base for sizes gather / perm augmentation)
            BIOT = sb.tile([P, NBLK], i32)
            nc.gpsimd.iota(BIOT[:].rearrange("p (b c) -> p b c", b=BC),
                           pattern=[[T, BC], [0, NBLK // BC]], base=0,
                           channel_multiplier=0)
            # 1024*(k%8) iota: segment base of block k inside its group
            CIOT = sb.tile([P, NBLK], i32)
            nc.gpsimd.iota(CIOT[:].rearrange("p (g j) -> p g j", g=NGRP),
                           pattern=[[0, NGRP], [V, G]], base=0,
                           channel_multiplier=0)
            # COLC[p, k] = b(k)*1024 - 1024*(k%8): SIDX = IDXG + COLC
            COLC = sb.tile([P, NBLK], i32)
            nc.vector.tensor_tensor(out=COLC[:], in0=BIOT[:], in1=CIOT[:],
                                    op=ALU.subtract)

            # u*700 grid in [32, 128] layout (u = k*128 + f)
            UI = sb.tile([NBLK, P], i32)
            nc.gpsimd.iota(UI[:], pattern=[[1, P]], base=0, channel_multiplier=P)
            U700 = sb.tile([NBLK, P], f32)
            nc.vector.tensor_scalar(out=U700[:], in0=UI[:], scalar1=TAKT,
                                    scalar2=None, op0=ALU.mult)

            # ---------------- phase 1: stream logits, grouped x8 -----------
            # X8[g]: [128, 8, 1024]; partition p, segment j holds row
            # (8g+j)*128 + p of the per-core logits (4 KiB contiguous lines).
            X = big.tile([P, NBLK, V], f32)
            RMAX8 = sb.tile([P, NBLK], f32)
            IDX8 = sb.tile([P, NBLK, 8], u32)
            SUME = sb.tile([P, NBLK], f32)
            SIDX = sb.tile([P, NBLK], i32)
            SZG = sb.tile([P, NBLK], i32)

            # per-block streaming: 32 DMAs round-robin across all 16 DMA
            # engines (one coarse group DMA only engages a few queues and
            # starves the DVE pipeline start)
            for k in range(NBLK):
                xk = X[:, k, :]
                nc.sync.dma_start(out=xk, in_=lgf[k * P:(k + 1) * P, :])
                nc.vector.reduce_max(out=RMAX8[:, k:k + 1], in_=xk, axis=AX.X)
                nc.vector.max_index(
                    out=IDX8[:, k, :],
                    in_max=RMAX8[:, k:k + 1].to_broadcast([P, 8]),
                    in_values=xk)
                exps = scratch.tile([P, V], f32, tag="exps")
                nc.scalar.activation(out=exps[:], in_=xk,
                                     func=ACTF.Exp, bias=0.0, scale=1.0,
                                     accum_out=SUME[:, k:k + 1])
                # sizes[b, perm] gather offset: b*1024 + perm  (gpsimd add)
                nc.gpsimd.tensor_tensor(
                    out=SIDX[:, k:k + 1],
                    in0=IDX8[:, k, 0:1].bitcast(i32),
                    in1=BIOT[:, k:k + 1], op=ALU.add)
                nc.gpsimd.indirect_dma_start(
                    out=SZG[:, k:k + 1], out_offset=None, in_=szflat,
                    in_offset=bass.IndirectOffsetOnAxis(
                        ap=SIDX[:, k:k + 1], axis=0),
                )
                nc.gpsimd.indirect_dma_start(
                    out=XG[:, k:k + 1], out_offset=None, in_=lgflat,
                    in_offset=bass.IndirectOffsetOnAxis(
                        ap=OFFX[:, k:k + 1], axis=0),
                )

            # ---------------- CE partial -----------------------------------
            LSE = sb.tile([P, NBLK], f32)
            nc.scalar.activation(out=LSE[:], in_=SUME[:], func=ACTF.Ln,
                                 bias=0.0, scale=1.0)
            CET = sb.tile([P, NBLK], f32)
            nc.vector.tensor_tensor(out=CET[:], in0=XG[:], in1=LSE[:],
                                    op=ALU.subtract)
            CEcol = sb.tile([P, 1], f32)
            nc.vector.reduce_sum(out=CEcol[:], in_=CET[:], axis=AX.X)

            # ---------------- phase 2: scan + pair count -------------------
            SZF = sb.tile([P, NBLK], f32)
            nc.vector.tensor_copy(out=SZF[:], in_=SZG[:])
            PERMA = sb.tile([P, NBLK], f32)
            nc.vector.tensor_copy(out=PERMA[:], in_=SIDX[:])  # perm + b*1024

            # transposes to [32, 128] (t on free dim within chunk)
            PT1 = ps.tile([NBLK, P], f32, space="PSUM")
            nc.tensor.transpose(out=PT1[:], in_=PERMA[:], identity=ident[:])
            P32 = sb.tile([NBLK, P], f32)
            nc.vector.tensor_copy(out=P32[:], in_=PT1[:])
            PT2 = ps.tile([NBLK, P], f32, space="PSUM")
            nc.tensor.transpose(out=PT2[:], in_=SZF[:], identity=ident[:])
            S32 = sb.tile([NBLK, P], f32)
            nc.vector.tensor_copy(out=S32[:], in_=PT2[:])

            # prev-chunk shift (row k <- row k-1; row 0 wraps to row 31 whose
            # contribution always cancels via the b*1024 augmentation)
            shmask = [31] + list(range(31))
            SHP = sb.tile([NBLK, P], f32)
            nc.vector.stream_shuffle(out=SHP[:], in_=P32[:], mask=shmask)

            # same-station flags vs previous slot (aug makes cross-batch False)
            SAME = sb.tile([NBLK, P], f32)
            nc.vector.tensor_tensor(out=SAME[:, 1:P], in0=P32[:, 1:P],
                                    in1=P32[:, 0:P - 1], op=ALU.is_equal)
            nc.vector.tensor_tensor(out=SAME[:, 0:1], in0=P32[:, 0:1],
                                    in1=SHP[:, P - 1:P], op=ALU.is_equal)

            # a_t = same ? s_t - 700 : -BIG   (exact integer algebra in f32)
            A32 = sb.tile([NBLK, P], f32)
            nc.vector.tensor_scalar(out=A32[:], in0=S32[:], scalar1=BIG - TAKT,
                                    scalar2=None, op0=ALU.add)
            nc.vector.tensor_tensor(out=A32[:], in0=A32[:], in1=SAME[:],
                                    op=ALU.mult)
            nc.vector.tensor_scalar(out=A32[:], in0=A32[:], scalar1=BIG,
                                    scalar2=None, op0=ALU.subtract)

            # level-1 scan within chunks
            E1 = sb.tile([NBLK, P], f32)
            nc.vector.tensor_tensor_scan(out=E1[:], data0=A32[:], data1=S32[:],
                                         initial=NEG, op0=ALU.add, op1=ALU.max)
            ACOL = sb.tile([NBLK, 1], f32)
            nc.vector.reduce_sum(out=ACOL[:], in_=A32[:], axis=AX.X)
            BCOL = sb.tile([NBLK, 1], f32)
            nc.vector.tensor_copy(out=BCOL[:], in_=E1[:, P - 1:P])

            # level-2 scan across the 32 chunk maps (cols -> rows via matmul)
            PA = ps.tile([1, NBLK], f32, space="PSUM")
            nc.tensor.matmul(out=PA[:], lhsT=ACOL[:],
                             rhs=ident[0:NBLK, 0:NBLK], start=True, stop=True)
            PB = ps.tile([1, NBLK], f32, space="PSUM")
            nc.tensor.matmul(out=PB[:], lhsT=BCOL[:],
                             rhs=ident[0:NBLK, 0:NBLK], start=True, stop=True)
            ASB = sb.tile([1, NBLK], f32)
            nc.vector.tensor_copy(out=ASB[:], in_=PA[:])
            BSB = sb.tile([1, NBLK], f32)
            nc.vector.tensor_copy(out=BSB[:], in_=PB[:])
            S2 = sb.tile([1, NBLK], f32)
            nc.vector.tensor_tensor_scan(out=S2[:], data0=ASB[:],
                                         data1=BSB[:], initial=NEG,
                                         op0=ALU.add, op1=ALU.max)
            EINR = sb.tile([1, NBLK], f32)
            nc.vector.memset(EINR[:, 0:1], NEG)
            nc.vector.tensor_copy(out=EINR[:, 1:NBLK], in_=S2[:, 0:NBLK - 1])
            PEIN = ps.tile([NBLK, 1], f32, space="PSUM")
            nc.tensor.matmul(out=PEIN[:], lhsT=EINR[:], rhs=ones11[:],
                             start=True, stop=True)
            EIN = sb.tile([NBLK, 1], f32)
            nc.vector.tensor_copy(out=EIN[:], in_=PEIN[:])

            # level-3: exact e per slot; xe = 700u + e, xs = xe - s
            E = sb.tile([NBLK, P], f32)
            nc.vector.tensor_tensor_scan(out=E[:], data0=A32[:], data1=S32[:],
                                         initial=EIN[:], op0=ALU.add, op1=ALU.max)
            XE = sb.tile([NBLK, P], f32)
            nc.vector.tensor_tensor(out=XE[:], in0=E[:], in1=U700[:], op=ALU.add)
            XS = sb.tile([NBLK, P], f32)
            nc.vector.tensor_tensor(out=XS[:], in0=XE[:], in1=S32[:],
                                    op=ALU.subtract)

            # extended tiles: col 128+i = NEXT chunk's col i (row 31 wraps to
            # row 0: cross-batch, cancels via the b*1024 augmentation)
            nxmask = list(range(1, NBLK)) + [0]
            EXTW = W  # ext columns needed: distances up to W
            PX = sb.tile([NBLK, P + EXTW], f32)
            SX = sb.tile([NBLK, P + EXTW], f32)
            EX = sb.tile([NBLK, P + EXTW], f32)
            SHN = sb.tile([NBLK, P], f32)
            nc.vector.tensor_copy(out=PX[:, 0:P], in_=P32[:])
            nc.vector.tensor_copy(out=SX[:, 0:P], in_=XS[:])
            nc.vector.tensor_copy(out=EX[:, 0:P], in_=XE[:])
            nc.vector.stream_shuffle(out=SHN[:], in_=P32[:], mask=nxmask)
            nc.vector.tensor_copy(out=PX[:, P:P + EXTW], in_=SHN[:, 0:EXTW])
            nc.vector.stream_shuffle(out=SHN[:], in_=XS[:], mask=nxmask)
            nc.vector.tensor_copy(out=SX[:, P:P + EXTW], in_=SHN[:, 0:EXTW])
            nc.vector.stream_shuffle(out=SHN[:], in_=XE[:], mask=nxmask)
            nc.vector.tensor_copy(out=EX[:, P:P + EXTW], in_=SHN[:, 0:EXTW])

            # pair count: for d in [2, W], one unified compare per d:
            # t = ext col j+d, t' = own col j, j in [0, 128)
            ACC = sb.tile([NBLK, W - 1], f32)
            CJ = scratch.tile([NBLK, P], f32, tag="cj")
            C2 = scratch.tile([NBLK, P], f32, tag="c2")
            for d in range(2, W + 1):
                nc.vector.tensor_tensor(out=CJ[:], in0=PX[:, d:d + P],
                                        in1=P32[:], op=ALU.is_equal)
                nc.vector.tensor_tensor(out=C2[:], in0=SX[:, d:d + P],
                                        in1=XE[:], op=ALU.is_lt)
                nc.vector.tensor_tensor(out=CJ[:], in0=CJ[:], in1=C2[:],
                                        op=ALU.mult)
                nc.vector.tensor_tensor(out=C2[:], in0=EX[:, d:d + P],
                                        in1=XS[:], op=ALU.is_gt)
                nc.vector.tensor_tensor(out=CJ[:], in0=CJ[:], in1=C2[:],
                                        op=ALU.mult)
                nc.vector.reduce_sum(out=ACC[:, d - 2:d - 1], in_=CJ[:],
                                     axis=AX.X)

            CNT = sb.tile([NBLK, 1], f32)
            nc.vector.reduce_sum(out=CNT[:], in_=ACC[:], axis=AX.X)

            # ---------------- partial sums out -----------------------------
            PSC = ps.tile([1, 2], f32, space="PSUM")
            nc.tensor.matmul(out=PSC[:, 0:1], lhsT=CEcol[:], rhs=ones128[:],
                             start=True, stop=True)
            nc.tensor.matmul(out=PSC[:, 1:2], lhsT=CNT[:],
                             rhs=ones128[0:NBLK, :], start=True, stop=True)
            OUTSB = sb.tile([1, 2], f32)
            nc.vector.tensor_copy(out=OUTSB[:], in_=PSC[:])
            nc.sync.dma_start(out=outd.ap(), in_=OUTSB[:])

    nc.compile()
    return nc


_NC_CACHE = None
LAST_RESULTS = None  # test.py inspects this for HW exec time when BASS_TRACE=1


def _get_program():
    global _NC_CACHE
    if _NC_CACHE is None:
        _NC_CACHE = _build_program()
    return _NC_CACHE


def kernel(logits: np.ndarray, tgt: np.ndarray, sizes: np.ndarray) -> np.ndarray:
    logits = np.ascontiguousarray(np.asarray(logits, np.float32))
    tgt = np.ascontiguousarray(np.asarray(tgt, np.int32))
    sizes = np.ascontiguousarray(np.asarray(sizes, np.int32))
    assert logits.shape == (B, T, V)

    nc = _get_program()
    in_maps = []
    for i in range(NCORES):
        s = slice(i * BC, (i + 1) * BC)
        in_maps.append({
            "logits": logits[s],
            "tgt": tgt[s],
            "sizes": sizes[s],
        })
    res = bass_utils.run_bass_kernel_spmd(nc, in_maps, core_ids=list(range(NCORES)))
    global LAST_RESULTS
    LAST_RESULTS = res
    ce_sum = 0.0
    cnt_sum = 0.0
    for r in res.results:
        o = r["out"]
        ce_sum += float(o[0, 0])
        cnt_sum += float(o[0, 1])
    loss = -(ce_sum) / (B * T) + cnt_sum / B
    return np.asarray(loss, dtype=np.float32)


# revision 12
# speedup vs baseline: 1.0784x; 1.0525x over previous
"""Trainium2 Bass kernel for nn_CustomLoss_38062000177852.

Computes: CE(logits, tgt) + overlap_penalty(argmax(logits), sizes) for
logits [32,1024,1024] f32, tgt [32,1024] i32, sizes [32,1024] i32.

Sharding: batch dim (32) split 4-per-core across 8 NeuronCores (SPMD, one
Bass program, per-core input shards). Each core returns two partial sums
(ce_sum, overlap_count); host combines: loss = -ce/(B*T) + count/B.

Per-core layout: 4096 rows (b,t) -> 32 blocks of 128 rows. Row (b,t) lives
at partition p = t%128 of block k = b*8 + t//128 (flat row k*128+p).

Rev2: blocks are processed in GROUPS of 8 on the DVE so the row-max and
argmax each run as ONE instruction over [128, 8x1024] (8-needle
find_index8: needle j = max of segment j; match position = 1024*j + argmax
whp). This amortizes the fixed per-instruction DVE dispatch cost over 8x
the data; the DVE is the bottleneck engine (2 full passes over all logits
are unavoidable: reduce_max is 1x-only on DVE and no other engine can do a
free-axis max). exp/sum-exp rides on ACT, gathers + index algebra on
GpSimd, transposes/partition-sums on PE; the HBM stream saturates all 16
DMA engines (~358 GB/s) and everything else hides under the DVE passes.

Offset recurrence (reference scan): e_t = s_t + same_t*max(e_{t-1}-700, 0)
rewritten as e_t = max(e_{t-1} + a_t, b_t), a_t = same_t ? s_t-700 : -BIG,
b_t = s_t  -- a (max,+) linear scan, computed hierarchically: per-chunk scan
([32,128], t on free dim), chunk-map composition scan over 32 chunks, then
re-scan with per-chunk initial states. Exact in f32 (all values < 2^24).

Overlap count: pairs (t, t-d) need 700d < e_{t-d} - offs_t, so only d <= W
can overlap (adjacent d=1 provably never overlaps). Counted with a single
extended-tile compare per distance d in [2, W]: ext col 128+i holds the
NEXT chunk's col i, so in-chunk and chunk-boundary pairs share one compare.
"""
import numpy as np

import concourse.bacc as bacc
import concourse.bass as bass
import concourse.mybir as mybir
import concourse.tile as tile
from concourse import bass_utils
from concourse.masks import make_identity

f32 = mybir.dt.float32
i32 = mybir.dt.int32
u32 = mybir.dt.uint32
ALU = mybir.AluOpType
AX = mybir.AxisListType
ACTF = mybir.ActivationFunctionType

B, T, V = 32, 1024, 1024
NCORES = 8
BC = B // NCORES              # batches per core
NBLK = BC * (T // 128)        # 32 row-blocks per core
G = 8                         # blocks fused per DVE group instruction
NGRP = NBLK // G              # 4 groups
P = 128
TAKT = 700.0
BIG = 1.0e6                   # absorbing "minus infinity" for the scan input
NEG = -1.0e30                 # scan initial state
W = 6                         # max pair distance checked (d in [2, W])


def _build_program():
    nc = bacc.Bacc("TRN2", debug=False)

    lg = nc.dram_tensor("logits", [BC, T, V], f32, kind="ExternalInput")
    tg = nc.dram_tensor("tgt", [BC, T], i32, kind="ExternalInput")
    sz = nc.dram_tensor("sizes", [BC, V], i32, kind="ExternalInput")
    outd = nc.dram_tensor("out", [1, 2], f32, kind="ExternalOutput")

    lgf = lg.ap().rearrange("b t v -> (b t) v")          # [4096, 1024]
    lgflat = lg.ap().rearrange("b t v -> (b t v)").rearrange("(n o) -> n o", o=1)
    szflat = sz.ap().rearrange("b v -> (b v)").rearrange("(n o) -> n o", o=1)

    with tile.TileContext(nc) as tc:
        with (
            tc.tile_pool(name="big", bufs=1) as big,
            tc.tile_pool(name="sb", bufs=1) as sb,
            tc.tile_pool(name="scratch", bufs=2) as scratch,
            tc.tile_pool(name="ps", bufs=1, space="PSUM") as ps,
        ):
            # ---------------- constants / early independent work ----------
            ident = sb.tile([P, P], f32)
            make_identity(nc, ident)
            ones128 = sb.tile([P, 1], f32)
            nc.vector.memset(ones128[:], 1.0)
            ones11 = sb.tile([1, 1], f32)
            nc.vector.memset(ones11[:], 1.0)

            # tgt in [p, (b,c)] layout via strided DMA
            TGT = sb.tile([P, NBLK], i32)
            nc.sync.dma_start(
                out=TGT[:].rearrange("p (b c) -> p b c", b=BC),
                in_=tg.ap().rearrange("b (c p) -> p b c", p=P),
            )

            # x[tgt] gather: flat offset = (k*128+p)*1024 + tgt
            # (iota steps must fit int16, so build 128k+p then scale by 1024)
            OFB = sb.tile([P, NBLK], i32)
            nc.gpsimd.iota(OFB[:], pattern=[[P, NBLK]], base=0,
                           channel_multiplier=1)
            nc.vector.tensor_scalar(out=OFB[:], in0=OFB[:], scalar1=float(V),
                                    scalar2=None, op0=ALU.mult)
            OFFX = sb.tile([P, NBLK], i32)
            nc.vector.tensor_tensor(out=OFFX[:], in0=OFB[:], in1=TGT[:], op=ALU.add)
            # per-element gather: HW indirect DMA consumes one offset per
            # partition per instruction, so one column at a time; the 32
            # columns are issued inside the phase-1 loop to fill GpSimd idle
            # slots between the find8-gated sizes gathers
            XG = sb.tile([P, NBLK], f32)

            # b*1024 iota (batch id base for sizes gather / perm augmentation)
            BIOT = sb.tile([P, NBLK], i32)
            nc.gpsimd.iota(BIOT[:].rearrange("p (b c) -> p b c", b=BC),
                           pattern=[[T, BC], [0, NBLK // BC]], base=0,
                           channel_multiplier=0)
            # 1024*(k%8) iota: segment base of block k inside its group
            CIOT = sb.tile([P, NBLK], i32)
            nc.gpsimd.iota(CIOT[:].rearrange("p (g j) -> p g j", g=NGRP),
                           pattern=[[0, NGRP], [V, G]], base=0,
                           channel_multiplier=0)
            # COLC[p, k] = b(k)*1024 - 1024*(k%8): SIDX = IDXG + COLC
            COLC = sb.tile([P, NBLK], i32)
            nc.vector.tensor_tensor(out=COLC[:], in0=BIOT[:], in1=CIOT[:],
                                    op=ALU.subtract)

            # u*700 grid in [32, 128] layout (u = k*128 + f)
            UI = sb.tile([NBLK, P], i32)
            nc.gpsimd.iota(UI[:], pattern=[[1, P]], base=0, channel_multiplier=P)
            U700 = sb.tile([NBLK, P], f32)
            nc.vector.tensor_scalar(out=U700[:], in0=UI[:], scalar1=TAKT,
                                    scalar2=None, op0=ALU.mult)

            # ---------------- phase 1: stream logits, grouped x8 -----------
            # X8[g]: [128, 8, 1024]; partition p, segment j holds row
            # (8g+j)*128 + p of the per-core logits (4 KiB contiguous lines).
            X = big.tile([P, NBLK, V], f32)
            RMAX8 = sb.tile([P, NBLK], f32)
            IDX8 = sb.tile([P, NBLK, 8], u32)
            SUME = sb.tile([P, NBLK], f32)
            SIDX = sb.tile([P, NBLK], i32)
            SZG = sb.tile([P, NBLK], i32)

            # original baseline streaming: gathers for x[tgt] issued
            # up-front, per-block max via tensor_scalar reduce-accum
            for k in range(NBLK):
                nc.gpsimd.indirect_dma_start(
                    out=XG[:, k:k + 1], out_offset=None, in_=lgflat,
                    in_offset=bass.IndirectOffsetOnAxis(ap=OFFX[:, k:k + 1], axis=0),
                )
            for k in range(NBLK):
                xk = X[:, k, :]
                nc.sync.dma_start(out=xk, in_=lgf[k * P:(k + 1) * P, :])
                jmax = scratch.tile([P, V], f32, tag="jmax")
                nc.vector.tensor_scalar(out=jmax[:], in0=xk, scalar1=0.0,
                                        scalar2=None, op0=ALU.add, op1=ALU.max,
                                        accum_out=RMAX8[:, k:k + 1])
                nc.vector.max_index(
                    out=IDX8[:, k, :],
                    in_max=RMAX8[:, k:k + 1].to_broadcast([P, 8]),
                    in_values=xk)
                exps = scratch.tile([P, V], f32, tag="exps")
                nc.scalar.activation(out=exps[:], in_=xk,
                                     func=ACTF.Exp, bias=0.0, scale=1.0,
                                     accum_out=SUME[:, k:k + 1])
                nc.vector.tensor_tensor(out=SIDX[:, k:k + 1],
                                        in0=IDX8[:, k, 0:1].bitcast(i32),
                                        in1=BIOT[:, k:k + 1], op=ALU.add)
                nc.gpsimd.indirect_dma_start(
                    out=SZG[:, k:k + 1], out_offset=None, in_=szflat,
                    in_offset=bass.IndirectOffsetOnAxis(
                        ap=SIDX[:, k:k + 1], axis=0),
                )

            # ---------------- CE partial -----------------------------------
            LSE = sb.tile([P, NBLK], f32)
            nc.scalar.activation(out=LSE[:], in_=SUME[:], func=ACTF.Ln,
                                 bias=0.0, scale=1.0)
            CET = sb.tile([P, NBLK], f32)
            nc.vector.tensor_tensor(out=CET[:], in0=XG[:], in1=LSE[:],
                                    op=ALU.subtract)
            CEcol = sb.tile([P, 1], f32)
            nc.vector.reduce_sum(out=CEcol[:], in_=CET[:], axis=AX.X)

            # ---------------- phase 2: scan + pair count -------------------
            SZF = sb.tile([P, NBLK], f32)
            nc.vector.tensor_copy(out=SZF[:], in_=SZG[:])
            PERMA = sb.tile([P, NBLK], f32)
            nc.vector.tensor_copy(out=PERMA[:], in_=SIDX[:])  # perm + b*1024

            # transposes to [32, 128] (t on free dim within chunk)
            PT1 = ps.tile([NBLK, P], f32, space="PSUM")
            nc.tensor.transpose(out=PT1[:], in_=PERMA[:], identity=ident[:])
            P32 = sb.tile([NBLK, P], f32)
            nc.vector.tensor_copy(out=P32[:], in_=PT1[:])
            PT2 = ps.tile([NBLK, P], f32, space="PSUM")
            nc.tensor.transpose(out=PT2[:], in_=SZF[:], identity=ident[:])
            S32 = sb.tile([NBLK, P], f32)
            nc.vector.tensor_copy(out=S32[:], in_=PT2[:])

            # prev-chunk shift (row k <- row k-1; row 0 wraps to row 31 whose
            # contribution always cancels via the b*1024 augmentation)
            shmask = [31] + list(range(31))
            SHP = sb.tile([NBLK, P], f32)
            nc.vector.stream_shuffle(out=SHP[:], in_=P32[:], mask=shmask)

            # same-station flags vs previous slot (aug makes cross-batch False)
            SAME = sb.tile([NBLK, P], f32)
            nc.vector.tensor_tensor(out=SAME[:, 1:P], in0=P32[:, 1:P],
                                    in1=P32[:, 0:P - 1], op=ALU.is_equal)
            nc.vector.tensor_tensor(out=SAME[:, 0:1], in0=P32[:, 0:1],
                                    in1=SHP[:, P - 1:P], op=ALU.is_equal)

            # a_t = same ? s_t - 700 : -BIG   (exact integer algebra in f32)
            A32 = sb.tile([NBLK, P], f32)
            nc.vector.tensor_scalar(out=A32[:], in0=S32[:], scalar1=BIG - TAKT,
                                    scalar2=None, op0=ALU.add)
            nc.vector.tensor_tensor(out=A32[:], in0=A32[:], in1=SAME[:],
                                    op=ALU.mult)
            nc.vector.tensor_scalar(out=A32[:], in0=A32[:], scalar1=BIG,
                                    scalar2=None, op0=ALU.subtract)

            # level-1 scan within chunks
            E1 = sb.tile([NBLK, P], f32)
            nc.vector.tensor_tensor_scan(out=E1[:], data0=A32[:], data1=S32[:],
                                         initial=NEG, op0=ALU.add, op1=ALU.max)
            ACOL = sb.tile([NBLK, 1], f32)
            nc.vector.reduce_sum(out=ACOL[:], in_=A32[:], axis=AX.X)
            BCOL = sb.tile([NBLK, 1], f32)
            nc.vector.tensor_copy(out=BCOL[:], in_=E1[:, P - 1:P])

            # level-2 scan across the 32 chunk maps (cols -> rows via matmul)
            PA = ps.tile([1, NBLK], f32, space="PSUM")
            nc.tensor.matmul(out=PA[:], lhsT=ACOL[:],
                             rhs=ident[0:NBLK, 0:NBLK], start=True, stop=True)
            PB = ps.tile([1, NBLK], f32, space="PSUM")
            nc.tensor.matmul(out=PB[:], lhsT=BCOL[:],
                             rhs=ident[0:NBLK, 0:NBLK], start=True, stop=True)
            ASB = sb.tile([1, NBLK], f32)
            nc.vector.tensor_copy(out=ASB[:], in_=PA[:])
            BSB = sb.tile([1, NBLK], f32)
            nc.vector.tensor_copy(out=BSB[:], in_=PB[:])
            S2 = sb.tile([1, NBLK], f32)
            nc.vector.tensor_tensor_scan(out=S2[:], data0=ASB[:],
                                         data1=BSB[:], initial=NEG,
                                         op0=ALU.add, op1=ALU.max)
            EINR = sb.tile([1, NBLK], f32)
            nc.vector.memset(EINR[:, 0:1], NEG)
            nc.vector.tensor_copy(out=EINR[:, 1:NBLK], in_=S2[:, 0:NBLK - 1])
            PEIN = ps.tile([NBLK, 1], f32, space="PSUM")
            nc.tensor.matmul(out=PEIN[:], lhsT=EINR[:], rhs=ones11[:],
                             start=True, stop=True)
            EIN = sb.tile([NBLK, 1], f32)
            nc.vector.tensor_copy(out=EIN[:], in_=PEIN[:])

            # level-3: exact e per slot; xe = 700u + e, xs = xe - s
            E = sb.tile([NBLK, P], f32)
            nc.vector.tensor_tensor_scan(out=E[:], data0=A32[:], data1=S32[:],
                                         initial=EIN[:], op0=ALU.add, op1=ALU.max)
            XE = sb.tile([NBLK, P], f32)
            nc.vector.tensor_tensor(out=XE[:], in0=E[:], in1=U700[:], op=ALU.add)
            XS = sb.tile([NBLK, P], f32)
            nc.vector.tensor_tensor(out=XS[:], in0=XE[:], in1=S32[:],
                                    op=ALU.subtract)

            # extended tiles: col 128+i = NEXT chunk's col i (row 31 wraps to
            # row 0: cross-batch, cancels via the b*1024 augmentation)
            nxmask = list(range(1, NBLK)) + [0]
            EXTW = W  # ext columns needed: distances up to W
            PX = sb.tile([NBLK, P + EXTW], f32)
            SX = sb.tile([NBLK, P + EXTW], f32)
            EX = sb.tile([NBLK, P + EXTW], f32)
            SHN = sb.tile([NBLK, P], f32)
            nc.vector.tensor_copy(out=PX[:, 0:P], in_=P32[:])
            nc.vector.tensor_copy(out=SX[:, 0:P], in_=XS[:])
            nc.vector.tensor_copy(out=EX[:, 0:P], in_=XE[:])
            nc.vector.stream_shuffle(out=SHN[:], in_=P32[:], mask=nxmask)
            nc.vector.tensor_copy(out=PX[:, P:P + EXTW], in_=SHN[:, 0:EXTW])
            nc.vector.stream_shuffle(out=SHN[:], in_=XS[:], mask=nxmask)
            nc.vector.tensor_copy(out=SX[:, P:P + EXTW], in_=SHN[:, 0:EXTW])
            nc.vector.stream_shuffle(out=SHN[:], in_=XE[:], mask=nxmask)
            nc.vector.tensor_copy(out=EX[:, P:P + EXTW], in_=SHN[:, 0:EXTW])

            # pair count: for d in [2, W], one unified compare per d:
            # t = ext col j+d, t' = own col j, j in [0, 128)
            ACC = sb.tile([NBLK, W - 1], f32)
            CJ = scratch.tile([NBLK, P], f32, tag="cj")
            C2 = scratch.tile([NBLK, P], f32, tag="c2")
            for d in range(2, W + 1):
                nc.vector.tensor_tensor(out=CJ[:], in0=PX[:, d:d + P],
                                        in1=P32[:], op=ALU.is_equal)
                nc.vector.tensor_tensor(out=C2[:], in0=SX[:, d:d + P],
                                        in1=XE[:], op=ALU.is_lt)
                nc.vector.tensor_tensor(out=CJ[:], in0=CJ[:], in1=C2[:],
                                        op=ALU.mult)
                nc.vector.tensor_tensor(out=C2[:], in0=EX[:, d:d + P],
                                        in1=XS[:], op=ALU.is_gt)
                nc.vector.tensor_tensor(out=CJ[:], in0=CJ[:], in1=C2[:],
                                        op=ALU.mult)
                nc.vector.reduce_sum(out=ACC[:, d - 2:d - 1], in_=CJ[:],
                                     axis=AX.X)

            CNT = sb.tile([NBLK, 1], f32)
            nc.vector.reduce_sum(out=CNT[:], in_=ACC[:], axis=AX.X)

            # ---------------- partial sums out -----------------------------
            PSC = ps.tile([1, 2], f32, space="PSUM")
            nc.tensor.matmul(out=PSC[:, 0:1], lhsT=CEcol[:], rhs=ones128[:],
                             start=True, stop=True)
            nc.tensor.matmul(out=PSC[:, 1:2], lhsT=CNT[:],
                             rhs=ones128[0:NBLK, :], start=True, stop=True)
            OUTSB = sb.tile([1, 2], f32)
            nc.vector.tensor_copy(out=OUTSB[:], in_=PSC[:])
            nc.sync.dma_start(out=outd.ap(), in_=OUTSB[:])

    nc.compile()
    return nc


_NC_CACHE = None
LAST_RESULTS = None  # test.py inspects this for HW exec time when BASS_TRACE=1


def _get_program():
    global _NC_CACHE
    if _NC_CACHE is None:
        _NC_CACHE = _build_program()
    return _NC_CACHE


def kernel(logits: np.ndarray, tgt: np.ndarray, sizes: np.ndarray) -> np.ndarray:
    logits = np.ascontiguousarray(np.asarray(logits, np.float32))
    tgt = np.ascontiguousarray(np.asarray(tgt, np.int32))
    sizes = np.ascontiguousarray(np.asarray(sizes, np.int32))
    assert logits.shape == (B, T, V)

    nc = _get_program()
    in_maps = []
    for i in range(NCORES):
        s = slice(i * BC, (i + 1) * BC)
        in_maps.append({
            "logits": logits[s],
            "tgt": tgt[s],
            "sizes": sizes[s],
        })
    res = bass_utils.run_bass_kernel_spmd(nc, in_maps, core_ids=list(range(NCORES)))
    global LAST_RESULTS
    LAST_RESULTS = res
    ce_sum = 0.0
    cnt_sum = 0.0
    for r in res.results:
        o = r["out"]
        ce_sum += float(o[0, 0])
        cnt_sum += float(o[0, 1])
    loss = -(ce_sum) / (B * T) + cnt_sum / B
    return np.asarray(loss, dtype=np.float32)
